# revision 29
# baseline (speedup 1.0000x reference)
"""Trainium2 Bass kernel for NNBlendFM: 3-layer tanh MLP embedder + 64-head
rank-16 factorization machine, data-parallel over batch across 8 NeuronCores.

Math (per batch row b, head h):
    h = tanh(tanh(tanh(x W1 + b1) W2 + b2) W3 + b3)          # [B, 2048]
    lin[b,h]  = h . fm_w[h]
    vx[b,h,r] = h . fm_V[h,r]
    diag[b,h] = (h*h) . (sum_r fm_V[h,r]^2)
    out[h,b]  = fm_w0[h] + lin + 0.5*(sum_r vx^2 - diag)

Device layout: activations kept as [feature_partition, batch_free] tiles so
every matmul contracts over the partition dim with natural-layout weights as
the stationary operand.  The FM stage flips to [batch_partition, col_free] by
using h^T k-tiles as the stationary operand.  All matmul inputs are bf16
(fp32 PSUM accumulation).

Schedule notes.  The kernel head is DMA-bound: the 16 DMA queues process
descriptors at ~25 GB/s each (~0.4 GB/us aggregate), per-queue FIFO, with
queue order = cross-engine issue-time order.  Before L2 can run, x (1 MiB)
+ W1 (2 MiB) + W2 (8 MiB) must land (~35 us), while L1 only has ~28 us of
compute to hide it.  Structural moves:

  * L2 and L3 each run as TWO k-half passes (kt 0-7, then kt 8-15).  Pass A
    drains its PSUM into a bf16 staging tile (zA); pass B accumulates the
    second half and a DVE add folds zA back into the PSUM before the tanh.
    L2-A therefore needs only W2's first half and starts right at L1-end
    (~41 us) instead of waiting for all of W2.  The split also relaxes
    every later weight-DMA deadline: W2's pass-A tiles free at L2-A end
    (~95 us), so the pool-ring slots for W3's last tiles and VT open two
    layers early -- no just-in-time stalls at L3/FM.
  * Issue order is strict: b1, then x-c0 + W1-q0 (gates L1), the rest of
    x/W1 in quarter pieces, then all of W2 serialized on the sync engine
    (nothing may wedge descriptors ahead of W2's first half).  W3's last
    four pair-DMAs and VT deliberately ring-block on pool slots -- a
    blocked dma_start enqueues no descriptors, keeping early queues clean.
  * L1 runs batch-chunk-outer (c0 jt0-15, then c1) so it can start on the
    first 1 MiB; 17 warm-up matmuls on a memset tile keep the PE busy from
    engine-start (~8.3 us) until L1's inputs land (~13.5 us), long enough
    that the HAM clock-gate releases (1.2 -> 2.4 GHz) during the warm-up
    and the whole kernel runs at full clock with zero PE gaps.
  * The last FM batch-tile computes vx in four 256-column chunks so its
    square+reduce pipeline drains during the matmuls, and per-tile output
    DMAs ship from the otherwise-idle gpsimd queue -- only ~1 us of
    reduce+DMA remains after the final matmul.

All weights are host-packed into [128, k*cols] order so SBUF tiles fill
with large contiguous DMAs; the output uses an SBUF-native [128, 8*64]
dram layout unpacked on the host (256 B dram lines would be slow).
"""

import numpy as np
import ml_dtypes

import concourse.tile as tile
from concourse import bacc, mybir
from concourse import bass_utils

BF16 = mybir.dt.bfloat16
F32 = mybir.dt.float32
AF = mybir.ActivationFunctionType
ALU = mybir.AluOpType

P = 128
IN, HID, HEADS, RANK = 512, 2048, 64, 16
B = 8192
NCORES = 8
BC = B // NCORES            # 1024 batch rows per core
KT1 = IN // P               # 4  k-tiles, layer 1
KT = HID // P               # 16 k-tiles, layers 2/3 + FM
KH = KT // 2                # 8  k-tiles per half pass
JT = HID // P               # 16 output-feature tiles per layer
NB = 512                    # matmul moving free-dim (one PSUM bank)
NBC = BC // NB              # 2 batch column chunks
BT = BC // P                # 8 batch tiles in FM stage
HR = HEADS * RANK           # 1024 vx columns
# HAM's activity window is free-running: un-throttle fires after up to TWO
# 3.4us windows of sustained busy.  17 matmuls = ~5.4us of continuous PE
# work from ~8.3us (8 cold + 9 warm), so the release lands during the
# warm-up block and the block drains at ~13.6us, just as L1's data-gated
# first groups become runnable -- measured gap-free and run-to-run stable.
WARMUP_MM = 17

_CACHE = {}


def _build_module():
    nc = bacc.Bacc(
        "TRN2", target_bir_lowering=False, debug=False, num_devices=NCORES
    )
    dt = nc.dram_tensor
    # host-packed layouts: [p, k*cols + c] = M[k*128 + p, c]
    XP = dt("XP", [P, KT1 * BC], BF16, kind="ExternalInput").ap()
    W1P = dt("W1P", [P, KT1 * HID], BF16, kind="ExternalInput").ap()
    W2P = dt("W2P", [P, KT * HID], BF16, kind="ExternalInput").ap()
    W3P = dt("W3P", [P, KT * HID], BF16, kind="ExternalInput").ap()
    VTP = dt("VTP", [P, KT * HR], BF16, kind="ExternalInput").ap()
    B1 = dt("B1", [P, JT], F32, kind="ExternalInput").ap()
    B2 = dt("B2", [P, JT], F32, kind="ExternalInput").ap()
    B3 = dt("B3", [P, JT], F32, kind="ExternalInput").ap()
    FW = dt("FW", [P, KT * HEADS], BF16, kind="ExternalInput").ap()
    SQ = dt("SQ", [P, KT * HEADS], BF16, kind="ExternalInput").ap()
    W0C = dt("W0C", [P, HEADS], BF16, kind="ExternalInput").ap()
    # SBUF-native layout [p, bt*64+c] = out[bt*128+p, c]; host unpacks.
    # (A row-major [1024, 64] target would need 256 B per-partition DMA
    # lines -- slow scattered descriptors right on the kernel tail.)
    OUT = dt("out", [P, BT * HEADS], F32, kind="ExternalOutput").ap()

    with tile.TileContext(nc) as tc:
        with (
            tc.tile_pool(name="wpool", bufs=12) as wpool,   # 12 x 8KiB
            tc.tile_pool(name="hpool", bufs=32) as hpool,   # 32 x 2KiB
            tc.tile_pool(name="zpool", bufs=JT) as zpool,   # 16 x 2KiB staging
            tc.tile_pool(name="cpool", bufs=1) as cpool,
            tc.tile_pool(name="pp", bufs=8, space="PSUM") as pp,
            tc.tile_pool(name="epool", bufs=2) as epool,
            tc.tile_pool(name="spool", bufs=8) as spool,
            tc.tile_pool(name="opool", bufs=1) as opool,
        ):
            # --- PE warm-up --------------------------------------------------
            warm = cpool.tile([P, NB], BF16, tag="warm")
            nc.vector.memset(warm[:], 0.0)
            wu = pp.tile([P, NB], F32, tag="ps", name="warm")
            for _ in range(WARMUP_MM):
                nc.tensor.matmul(
                    wu[:], warm[:, 0:P], warm[:], start=True, stop=True
                )

            # --- SBUF tiles (allocation order defines pool-ring reuse) -------
            # wpool ring (12 slots): w3p[0:2] take fresh slots, w3p[2:4] the
            # W1 slots (free ~40us), w3p[4:8] the W2 pass-A slots (~95us),
            # vtq[0:4] the W2 pass-B slots (~150us).  All arrive well before
            # their first reads (L3-A ~151, L3-B ~207, FM ~262).
            xt = [hpool.tile([P, BC], BF16, tag="h", name=f"xt{k}")
                  for k in range(KT1)]
            w1p = [wpool.tile([P, 2 * HID], BF16, tag="w", name=f"w1p{k}")
                   for k in range(KT1 // 2)]
            w2p = [wpool.tile([P, 2 * HID], BF16, tag="w", name=f"w2p{k}")
                   for k in range(KT // 2)]
            w3p = [wpool.tile([P, 2 * HID], BF16, tag="w", name=f"w3p{k}")
                   for k in range(KT // 2)]
            vtq = [wpool.tile([P, 4 * HR], BF16, tag="w", name=f"vtq{q}")
                   for q in range(KT // 4)]
            zA = [zpool.tile([P, BC], BF16, tag="z", name=f"zA{j}")
                  for j in range(JT)]

            def wsl(tiles, kt, j):
                """[128,128] stationary slice for k-tile kt, out-feature j."""
                base = (kt % 2) * HID + j * P
                return tiles[kt // 2][:, base: base + P]

            # --- DMA issue plan (sync/gpsimd/scalar only, ~0.7us each) ------
            # The head is DMA-QUEUE-rate bound (~25 GB/s per queue, 16 queues
            # ~= 0.4 GB/us) and per-queue FIFO order is the cross-engine
            # ISSUE-TIME order of each transfer's descriptors.  W1 streams in
            # column-QUARTERS (jt blocks of 4) so arrival paces L1's jt-order
            # consumption: 1 MiB (x c0 + W1 q0) gates the first matmul at
            # ~11us, and each later 0.5 MiB piece lands ~2us before its
            # first reader.  Nothing bulky may enqueue before the waves.
            b1t = cpool.tile([P, JT], F32, tag="b1")
            nc.scalar.dma_start(b1t[:], B1)           # needed by first ACT

            def w1q_dma(k, qr, eng):
                o = (k % 2) * HID + qr * NB
                eng.dma_start(
                    w1p[k // 2][:, o: o + NB],
                    W1P[:, k * HID + qr * NB: k * HID + (qr + 1) * NB],
                )

            # wave 1a: x chunk c0 + W1 quarter q0 (enables L1 c0 jt0-3)
            w1q_dma(0, 0, nc.scalar)
            nc.sync.dma_start(xt[0][:, 0:NB], XP[:, 0 * BC: 0 * BC + NB])
            nc.gpsimd.dma_start(xt[1][:, 0:NB], XP[:, 1 * BC: 1 * BC + NB])
            nc.scalar.dma_start(xt[2][:, 0:NB], XP[:, 2 * BC: 2 * BC + NB])
            nc.sync.dma_start(xt[3][:, 0:NB], XP[:, 3 * BC: 3 * BC + NB])
            w1q_dma(1, 0, nc.sync)
            w1q_dma(2, 0, nc.gpsimd)
            w1q_dma(3, 0, nc.gpsimd)
            # W1 q1 (jt4-7, needed ~14.5us)
            w1q_dma(0, 1, nc.scalar)
            w1q_dma(1, 1, nc.sync)
            w1q_dma(2, 1, nc.gpsimd)
            w1q_dma(3, 1, nc.gpsimd)
            # x c1 (needed ~25us)
            nc.sync.dma_start(xt[0][:, NB:BC], XP[:, 0 * BC + NB: 1 * BC])
            nc.gpsimd.dma_start(xt[1][:, NB:BC], XP[:, 1 * BC + NB: 2 * BC])
            nc.scalar.dma_start(xt[2][:, NB:BC], XP[:, 2 * BC + NB: 3 * BC])
            nc.sync.dma_start(xt[3][:, NB:BC], XP[:, 3 * BC + NB: 4 * BC])
            # W1 q2, q3 (jt8-15, needed ~18/21us)
            w1q_dma(0, 2, nc.scalar)
            w1q_dma(1, 2, nc.sync)
            w1q_dma(2, 2, nc.gpsimd)
            w1q_dma(3, 2, nc.gpsimd)
            w1q_dma(0, 3, nc.sync)
            w1q_dma(1, 3, nc.sync)
            w1q_dma(2, 3, nc.gpsimd)
            w1q_dma(3, 3, nc.gpsimd)

            # W2 entirely on sync, p0..p7 in order: its descriptors hit the
            # queues strictly first-half-first with nothing else wedged, so
            # p0-3 (gates L2-A at ~40us) complete ~27us.  gpsimd's bulk
            # (w3p4-7, vtq) all ring-block on pool slots that free at ~95us+
            # -- a blocked dma_start enqueues NO descriptors, keeping the
            # early queues clean.
            b2t = cpool.tile([P, JT], F32, tag="b2")
            nc.gpsimd.dma_start(b2t[:], B2)           # needed ~42us
            onest = cpool.tile([P, P], BF16, tag="ones")
            nc.gpsimd.memset(onest[:], 1.0)
            w0c = cpool.tile([P, HEADS], BF16, tag="w0c")
            nc.gpsimd.dma_start(w0c[:], W0C)
            for j in range(8):
                nc.sync.dma_start(w2p[j][:], W2P[:, j * 2 * HID: (j + 1) * 2 * HID])
            sqt = cpool.tile([P, KT * HEADS], BF16, tag="sq")
            nc.scalar.dma_start(sqt[:], SQ)
            b3t = cpool.tile([P, JT], F32, tag="b3")
            nc.sync.dma_start(b3t[:], B3)
            fwt = cpool.tile([P, KT * HEADS], BF16, tag="fw")
            nc.sync.dma_start(fwt[:], FW)
            # w3p0/p1 get fresh ring slots (descriptors follow W2's); p2/p3
            # block on sync until the W1 slots free (~44us); p4-7 block on
            # gpsimd until W2 pass-A slots free (~95us); vtq follows there.
            for j in range(4):
                nc.sync.dma_start(w3p[j][:], W3P[:, j * 2 * HID: (j + 1) * 2 * HID])
            for j in range(4, 8):
                nc.gpsimd.dma_start(w3p[j][:], W3P[:, j * 2 * HID: (j + 1) * 2 * HID])
            nc.gpsimd.dma_start(vtq[0][:], VTP[:, 0 * 4 * HR: 1 * 4 * HR])
            nc.gpsimd.dma_start(vtq[1][:], VTP[:, 1 * 4 * HR: 2 * 4 * HR])
            nc.gpsimd.dma_start(vtq[2][:], VTP[:, 2 * 4 * HR: 3 * 4 * HR])
            nc.gpsimd.dma_start(vtq[3][:], VTP[:, 3 * 4 * HR: 4 * 4 * HR])

            # --- embedder ----------------------------------------------------
            def layer1():
                """c-outer so chunk c0 only needs wave-1 data."""
                h_out = [hpool.tile([P, BC], BF16, tag="h", name=f"l1h{j}")
                         for j in range(JT)]
                for c in range(NBC):
                    for jt in range(JT):
                        nfill = 0
                        if c == 0 and nfill:
                            # dep-free fillers: execute only while the next
                            # group's trickling DMA pieces are late, keeping
                            # the PE busy so HAM doesn't re-throttle.
                            for _ in range(nfill):
                                nc.tensor.matmul(
                                    wu[:], warm[:, 0:P], warm[:],
                                    start=True, stop=True,
                                )
                        ps = pp.tile([P, NB], F32, tag="ps", name=f"l1ps{c}_{jt}")
                        kts = [(kt + jt) % KT1 for kt in range(KT1)]
                        for i, kt in enumerate(kts):
                            nc.tensor.matmul(
                                ps[:],
                                wsl(w1p, kt, jt),
                                xt[kt][:, c * NB: (c + 1) * NB],
                                start=(i == 0),
                                stop=(i == KT1 - 1),
                            )
                        nc.scalar.activation(
                            h_out[jt][:, c * NB: (c + 1) * NB],
                            ps[:],
                            AF.Tanh,
                            bias=b1t[:, jt: jt + 1],
                        )
                return h_out

            def layer2pass(h_prev, w_pairs, bias_t, name):
                """k-halved: pass A (kt 0-7) stages into bf16 zA, pass B
                (kt 8-15) accumulates, DVE folds zA in, ACT applies tanh."""
                # pass A
                for jt in range(JT):
                    ps = [pp.tile([P, NB], F32, tag="ps", name=f"{name}a{jt}_{c}")
                          for c in range(NBC)]
                    kts = [(kt + jt) % KH for kt in range(KH)]
                    for i, kt in enumerate(kts):
                        lhsT = wsl(w_pairs, kt, jt)
                        for c in range(NBC):
                            nc.tensor.matmul(
                                ps[c][:],
                                lhsT,
                                h_prev[kt][:, c * NB: (c + 1) * NB],
                                start=(i == 0),
                                stop=(i == KH - 1),
                            )
                    for c in range(NBC):
                        nc.vector.tensor_copy(
                            zA[jt][:, c * NB: (c + 1) * NB], ps[c][:]
                        )
                # pass B
                h_out = []
                for jt in range(JT):
                    ps = [pp.tile([P, NB], F32, tag="ps", name=f"{name}b{jt}_{c}")
                          for c in range(NBC)]
                    kts = [KH + (kt + jt) % KH for kt in range(KH)]
                    for i, kt in enumerate(kts):
                        lhsT = wsl(w_pairs, kt, jt)
                        for c in range(NBC):
                            nc.tensor.matmul(
                                ps[c][:],
                                lhsT,
                                h_prev[kt][:, c * NB: (c + 1) * NB],
                                start=(i == 0),
                                stop=(i == KH - 1),
                            )
                    ht = hpool.tile([P, BC], BF16, tag="h", name=f"{name}h{jt}")
                    for c in range(NBC):
                        nc.vector.tensor_add(
                            ps[c][:], ps[c][:], zA[jt][:, c * NB: (c + 1) * NB]
                        )
                        nc.scalar.activation(
                            ht[:, c * NB: (c + 1) * NB],
                            ps[c][:],
                            AF.Tanh,
                            bias=bias_t[:, jt: jt + 1],
                        )
                    h_out.append(ht)
                return h_out

            h1 = layer1()
            h2 = layer2pass(h1, w2p, b2t, "l2")
            h3 = layer2pass(h2, w3p, b3t, "l3")

            # --- h3 squared (stationary operand for the diag matmuls) -----
            h3sq = []
            for k in range(KT):
                sq_k = hpool.tile([P, BC], BF16, tag="h", name=f"h3sq{k}")
                nc.vector.tensor_mul(sq_k[:], h3[k][:], h3[k][:])
                h3sq.append(sq_k)

            # --- FM stage: per 128-row batch tile -------------------------
            def vsl(kt, half):
                """[128,512] moving slice of V^T for k-tile kt."""
                base = (kt % 4) * HR + half * NB
                return vtq[kt // 4][:, base: base + NB]

            def fm_phase_a(bt):
                """vx = h V^T (1024 cols) and lin = h fm_w^T (64 cols)."""
                vx0 = pp.tile([P, NB], F32, tag="ps", name=f"vx0_{bt}")
                vx1 = pp.tile([P, NB], F32, tag="ps", name=f"vx1_{bt}")
                lw = pp.tile([P, NB], F32, tag="ps", name=f"lw_{bt}")
                bsl = slice(bt * P, (bt + 1) * P)
                for kt in range(KT):
                    lhsT = h3[kt][:, bsl]
                    nc.tensor.matmul(
                        vx0[:], lhsT, vsl(kt, 0),
                        start=(kt == 0), stop=(kt == KT - 1),
                    )
                    nc.tensor.matmul(
                        vx1[:], lhsT, vsl(kt, 1),
                        start=(kt == 0), stop=(kt == KT - 1),
                    )
                    nc.tensor.matmul(
                        lw[:, 0:HEADS], lhsT,
                        fwt[:, kt * HEADS: (kt + 1) * HEADS],
                        start=(kt == 0), stop=(kt == KT - 1),
                    )
                return vx0, vx1, lw

            def fm_phase_b(bt):
                """diag = (h*h) . (0.5 * sum_r V^2), already scaled by 0.5."""
                dg = pp.tile([P, NB], F32, tag="ps", name=f"dg_{bt}")
                bsl = slice(bt * P, (bt + 1) * P)
                for kt in range(KT):
                    nc.tensor.matmul(
                        dg[:, 0:HEADS],
                        h3sq[kt][:, bsl],
                        sqt[:, kt * HEADS: (kt + 1) * HEADS],
                        start=(kt == 0), stop=False,
                    )
                nc.tensor.matmul(
                    dg[:, 0:HEADS], onest[:], w0c[:], start=False, stop=True,
                )
                return dg

            def fm_square_reduce(bt, vx0, vx1):
                """Emitted right after phase A: overlaps later bt's matmuls.
                Each 512-wide half squares then reduces independently so the
                two chains pipeline across ACT and DVE."""
                vx2 = epool.tile([P, HR], BF16, tag="e", name=f"vx2_{bt}")
                sumv = spool.tile([P, HEADS], F32, tag="s", name=f"sumv_{bt}")
                for c, vxh in ((0, vx0), (1, vx1)):
                    nc.scalar.activation(vx2[:, c * NB: (c + 1) * NB], vxh[:], AF.Square)
                    nc.vector.reduce_sum(
                        sumv[:, c * (HEADS // 2): (c + 1) * (HEADS // 2)],
                        vx2[:, c * NB: (c + 1) * NB].rearrange(
                            "p (h r) -> p h r", r=RANK
                        ),
                        axis=mybir.AxisListType.X,
                    )
                return sumv

            def fm_phase_a_chunked(bt):
                """Last batch tile only: vx in four 256-col chunks so each
                chunk's square+reduce drains DURING the next chunk's matmuls
                -- only ~0.6us of reduce work remains after the last matmul
                instead of the full 1.9us chain."""
                NCH = HR // 4
                lw = pp.tile([P, NB], F32, tag="ps", name=f"lw_{bt}")
                vx2 = epool.tile([P, HR], BF16, tag="e", name=f"vx2_{bt}")
                sumv = spool.tile([P, HEADS], F32, tag="s", name=f"sumv_{bt}")
                bsl = slice(bt * P, (bt + 1) * P)
                for ch in range(4):
                    vxc = pp.tile([P, NCH], F32, tag="ps", name=f"vxc{ch}_{bt}")
                    for kt in range(KT):
                        lhsT = h3[kt][:, bsl]
                        base = (kt % 4) * HR + ch * NCH
                        nc.tensor.matmul(
                            vxc[:], lhsT,
                            vtq[kt // 4][:, base: base + NCH],
                            start=(kt == 0), stop=(kt == KT - 1),
                        )
                        if ch == 0:
                            nc.tensor.matmul(
                                lw[:, 0:HEADS], lhsT,
                                fwt[:, kt * HEADS: (kt + 1) * HEADS],
                                start=(kt == 0), stop=(kt == KT - 1),
                            )
                    nc.scalar.activation(
                        vx2[:, ch * NCH: (ch + 1) * NCH], vxc[:], AF.Square
                    )
                    nc.vector.reduce_sum(
                        sumv[:, ch * (HEADS // 4): (ch + 1) * (HEADS // 4)],
                        vx2[:, ch * NCH: (ch + 1) * NCH].rearrange(
                            "p (h r) -> p h r", r=RANK
                        ),
                        axis=mybir.AxisListType.X,
                    )
                return sumv, lw

            ot = opool.tile([P, BT * HEADS], F32, tag="o")

            def fm_combine(bt, sumv, lw, dg):
                # q = 0.5*sumv - diag_half
                q = spool.tile([P, HEADS], F32, tag="s", name=f"q_{bt}")
                nc.vector.scalar_tensor_tensor(
                    q[:], sumv[:], 0.5, dg[:, 0:HEADS],
                    op0=ALU.mult, op1=ALU.subtract,
                )
                nc.vector.tensor_add(
                    ot[:, bt * HEADS: (bt + 1) * HEADS], q[:], lw[:, 0:HEADS]
                )
                # ship per-bt: gpsimd is idle through the FM stage, so only
                # the LAST tile's ~0.8us issue+transfer lands on the tail.
                nc.gpsimd.dma_start(
                    OUT[:, bt * HEADS: (bt + 1) * HEADS],
                    ot[:, bt * HEADS: (bt + 1) * HEADS],
                )

            # Stagger: A(0), A(1), B(0), C(0), A(2), B(1), C(1), ...
            pend = []  # (bt, sumv, lw)
            for bt in range(BT):
                if bt < BT - 1:
                    vx0, vx1, lw = fm_phase_a(bt)
                    sumv = fm_square_reduce(bt, vx0, vx1)
                else:
                    sumv, lw = fm_phase_a_chunked(bt)
                pend.append((bt, sumv, lw))
                if len(pend) == 2:
                    obt, osumv, olw = pend.pop(0)
                    dg = fm_phase_b(obt)
                    fm_combine(obt, osumv, olw, dg)
            while pend:
                obt, osumv, olw = pend.pop(0)
                dg = fm_phase_b(obt)
                fm_combine(obt, osumv, olw, dg)



    nc.compile()
    return nc


def _get_nc():
    if "nc" not in _CACHE:
        _CACHE["nc"] = _build_module()
    return _CACHE["nc"]


def _pack_rows(M, kt):
    """[kt*128, C] -> [128, kt*C] with [p, k*C+c] = M[k*128+p, c]."""
    C = M.shape[1]
    return np.ascontiguousarray(
        M.reshape(kt, P, C).transpose(1, 0, 2).reshape(P, kt * C)
    )


def _prep_host(x, W1, b1, W2, b2, W3, b3, fm_w0, fm_w, fm_V):
    """Host-side layout prep: bf16 casts, packing, per-head V reductions."""
    bf = ml_dtypes.bfloat16
    f32 = np.float32

    common = {
        "W1P": _pack_rows(W1.astype(bf), KT1),
        "W2P": _pack_rows(W2.astype(bf), KT),
        "W3P": _pack_rows(W3.astype(bf), KT),
        "B1": np.ascontiguousarray(b1.astype(f32).reshape(JT, P).T),
        "B2": np.ascontiguousarray(b2.astype(f32).reshape(JT, P).T),
        "B3": np.ascontiguousarray(b3.astype(f32).reshape(JT, P).T),
        # V^T: [2048, heads*rank] packed as [128, 16*1024]
        "VTP": _pack_rows(
            fm_V.reshape(HEADS * RANK, HID).T.astype(bf), KT
        ),
        # fm_w^T packed as [128, kt*64]: FW[p, kt*64+h] = fm_w[h, kt*128+p]
        "FW": np.ascontiguousarray(
            fm_w.T.reshape(KT, P, HEADS).transpose(1, 0, 2).reshape(P, KT * HEADS)
            .astype(bf)
        ),
        # 0.5 * sum_r V^2, same packing
        "SQ": np.ascontiguousarray(
            (0.5 * (fm_V.astype(np.float64) ** 2).sum(axis=1))
            .T.reshape(KT, P, HEADS).transpose(1, 0, 2).reshape(P, KT * HEADS)
            .astype(bf)
        ),
        "W0C": np.ascontiguousarray(
            np.tile((-fm_w0.astype(np.float64) / P)[None, :], (P, 1))
            .astype(ml_dtypes.bfloat16)
        ),
    }

    in_maps = []
    xb = x.astype(bf)
    for c in range(NCORES):
        m = dict(common)
        m["XP"] = _pack_rows(
            np.ascontiguousarray(xb[c * BC: (c + 1) * BC, :].T), KT1
        )
        in_maps.append(m)
    return in_maps


def kernel(x, W1, b1, W2, b2, W3, b3, fm_w0, fm_w, fm_V):
    # Host prep is plain numpy; coerce eagerly in case inputs are jax arrays.
    x, W1, b1, W2, b2, W3, b3, fm_w0, fm_w, fm_V = (
        np.asarray(a) for a in (x, W1, b1, W2, b2, W3, b3, fm_w0, fm_w, fm_V)
    )
    nc = _get_nc()
    in_maps = _prep_host(x, W1, b1, W2, b2, W3, b3, fm_w0, fm_w, fm_V)
    import os
    trace = bool(int(os.environ.get("KERNEL_TRACE", "0")))
    last_err = None
    for _attempt in range(3):
        try:
            res = bass_utils.run_bass_kernel_spmd(
                nc, in_maps, core_ids=list(range(NCORES)), trace=trace,
            )
            outs = [np.asarray(res.results[c]["out"]) for c in range(NCORES)]
            break
        except Exception as e:  # transient device faults (NRT unrecoverable)
            last_err = e
    else:
        raise last_err
    _CACHE["last_results"] = res
    # per-core out is [128, bt*64+c]; unpack to [BC, HEADS] then stack
    full = np.concatenate(
        [o.reshape(P, BT, HEADS).transpose(1, 0, 2).reshape(BC, HEADS)
         for o in outs],
        axis=0,
    )                                            # [B, HEADS]
    return np.ascontiguousarray(full.T).astype(np.float32)  # [HEADS, B]


# revision 30
# speedup vs baseline: 1.0051x; 1.0051x over previous
"""Trainium2 Bass kernel for NNBlendFM: 3-layer tanh MLP embedder + 64-head
rank-16 factorization machine, data-parallel over batch across 8 NeuronCores.

Math (per batch row b, head h):
    h = tanh(tanh(tanh(x W1 + b1) W2 + b2) W3 + b3)          # [B, 2048]
    lin[b,h]  = h . fm_w[h]
    vx[b,h,r] = h . fm_V[h,r]
    diag[b,h] = (h*h) . (sum_r fm_V[h,r]^2)
    out[h,b]  = fm_w0[h] + lin + 0.5*(sum_r vx^2 - diag)

Device layout: activations kept as [feature_partition, batch_free] tiles so
every matmul contracts over the partition dim with natural-layout weights as
the stationary operand.  The FM stage flips to [batch_partition, col_free] by
using h^T k-tiles as the stationary operand.  All matmul inputs are bf16
(fp32 PSUM accumulation).

Schedule notes.  The kernel head is DMA-bound: the 16 DMA queues process
descriptors at ~25 GB/s each (~0.4 GB/us aggregate), per-queue FIFO, with
queue order = cross-engine issue-time order.  Before L2 can run, x (1 MiB)
+ W1 (2 MiB) + W2 (8 MiB) must land (~35 us), while L1 only has ~28 us of
compute to hide it.  Structural moves:

  * L2 and L3 each run as TWO k-half passes (kt 0-7, then kt 8-15).  Pass A
    drains its PSUM into a bf16 staging tile (zA); pass B accumulates the
    second half and a DVE add folds zA back into the PSUM before the tanh.
    L2-A therefore needs only W2's first half and starts right at L1-end
    (~41 us) instead of waiting for all of W2.  The split also relaxes
    every later weight-DMA deadline: W2's pass-A tiles free at L2-A end
    (~95 us), so the pool-ring slots for W3's last tiles and VT open two
    layers early -- no just-in-time stalls at L3/FM.
  * Issue order is strict: b1, then x-c0 + W1-q0 (gates L1), the rest of
    x/W1 in quarter pieces, then all of W2 serialized on the sync engine
    (nothing may wedge descriptors ahead of W2's first half).  W3's last
    four pair-DMAs and VT deliberately ring-block on pool slots -- a
    blocked dma_start enqueues no descriptors, keeping early queues clean.
  * L1 runs batch-chunk-outer (c0 jt0-15, then c1) so it can start on the
    first 1 MiB; 17 warm-up matmuls on a memset tile keep the PE busy from
    engine-start (~8.3 us) until L1's inputs land (~13.5 us), long enough
    that the HAM clock-gate releases (1.2 -> 2.4 GHz) during the warm-up
    and the whole kernel runs at full clock with zero PE gaps.
  * The last FM batch-tile computes vx in four 256-column chunks so its
    square+reduce pipeline drains during the matmuls, and per-tile output
    DMAs ship from the otherwise-idle gpsimd queue -- only ~1 us of
    reduce+DMA remains after the final matmul.

All weights are host-packed into [128, k*cols] order so SBUF tiles fill
with large contiguous DMAs; the output uses an SBUF-native [128, 8*64]
dram layout unpacked on the host (256 B dram lines would be slow).
"""

import numpy as np
import ml_dtypes

import concourse.tile as tile
from concourse import bacc, mybir
from concourse import bass_utils

BF16 = mybir.dt.bfloat16
F32 = mybir.dt.float32
AF = mybir.ActivationFunctionType
ALU = mybir.AluOpType

P = 128
IN, HID, HEADS, RANK = 512, 2048, 64, 16
B = 8192
NCORES = 8
BC = B // NCORES            # 1024 batch rows per core
KT1 = IN // P               # 4  k-tiles, layer 1
KT = HID // P               # 16 k-tiles, layers 2/3 + FM
KH = KT // 2                # 8  k-tiles per half pass
JT = HID // P               # 16 output-feature tiles per layer
NB = 512                    # matmul moving free-dim (one PSUM bank)
NBC = BC // NB              # 2 batch column chunks
BT = BC // P                # 8 batch tiles in FM stage
HR = HEADS * RANK           # 1024 vx columns
# HAM's activity window is free-running: un-throttle fires after up to TWO
# 3.4us windows of sustained busy.  17 matmuls = ~5.4us of continuous PE
# work from ~8.3us (8 cold + 9 warm), so the release lands during the
# warm-up block and the block drains at ~13.6us, just as L1's data-gated
# first groups become runnable -- measured gap-free and run-to-run stable.
WARMUP_MM = 17

_CACHE = {}


def _build_module():
    nc = bacc.Bacc(
        "TRN2", target_bir_lowering=False, debug=False, num_devices=NCORES
    )
    dt = nc.dram_tensor
    # host-packed layouts: [p, k*cols + c] = M[k*128 + p, c]
    XP = dt("XP", [P, KT1 * BC], BF16, kind="ExternalInput").ap()
    W1P = dt("W1P", [P, KT1 * HID], BF16, kind="ExternalInput").ap()
    W2P = dt("W2P", [P, KT * HID], BF16, kind="ExternalInput").ap()
    W3P = dt("W3P", [P, KT * HID], BF16, kind="ExternalInput").ap()
    VTP = dt("VTP", [P, KT * HR], BF16, kind="ExternalInput").ap()
    B1 = dt("B1", [P, JT], F32, kind="ExternalInput").ap()
    B2 = dt("B2", [P, JT], F32, kind="ExternalInput").ap()
    B3 = dt("B3", [P, JT], F32, kind="ExternalInput").ap()
    FW = dt("FW", [P, KT * HEADS], BF16, kind="ExternalInput").ap()
    SQ = dt("SQ", [P, KT * HEADS], BF16, kind="ExternalInput").ap()
    W0C = dt("W0C", [P, HEADS], BF16, kind="ExternalInput").ap()
    # SBUF-native layout [p, bt*64+c] = out[bt*128+p, c]; host unpacks.
    # (A row-major [1024, 64] target would need 256 B per-partition DMA
    # lines -- slow scattered descriptors right on the kernel tail.)
    OUT = dt("out", [P, BT * HEADS], F32, kind="ExternalOutput").ap()

    with tile.TileContext(nc) as tc:
        with (
            tc.tile_pool(name="wpool", bufs=12) as wpool,   # 12 x 8KiB
            tc.tile_pool(name="hpool", bufs=32) as hpool,   # 32 x 2KiB
            tc.tile_pool(name="zpool", bufs=JT) as zpool,   # 16 x 2KiB staging
            tc.tile_pool(name="cpool", bufs=1) as cpool,
            tc.tile_pool(name="pp", bufs=8, space="PSUM") as pp,
            tc.tile_pool(name="epool", bufs=2) as epool,
            tc.tile_pool(name="spool", bufs=8) as spool,
            tc.tile_pool(name="opool", bufs=1) as opool,
        ):
            # --- PE warm-up --------------------------------------------------
            warm = cpool.tile([P, NB], BF16, tag="warm")
            nc.vector.memset(warm[:], 0.0)
            wu = pp.tile([P, NB], F32, tag="ps", name="warm")
            for _ in range(WARMUP_MM):
                nc.tensor.matmul(
                    wu[:], warm[:, 0:P], warm[:], start=True, stop=True
                )

            # --- SBUF tiles (allocation order defines pool-ring reuse) -------
            # wpool ring (12 slots): w3p[0:2] take fresh slots, w3p[2:4] the
            # W1 slots (free ~40us), w3p[4:8] the W2 pass-A slots (~95us),
            # vtq[0:4] the W2 pass-B slots (~150us).  All arrive well before
            # their first reads (L3-A ~151, L3-B ~207, FM ~262).
            xt = [hpool.tile([P, BC], BF16, tag="h", name=f"xt{k}")
                  for k in range(KT1)]
            w1p = [wpool.tile([P, 2 * HID], BF16, tag="w", name=f"w1p{k}")
                   for k in range(KT1 // 2)]
            w2p = [wpool.tile([P, 2 * HID], BF16, tag="w", name=f"w2p{k}")
                   for k in range(KT // 2)]
            w3p = [wpool.tile([P, 2 * HID], BF16, tag="w", name=f"w3p{k}")
                   for k in range(KT // 2)]
            vtq = [wpool.tile([P, 4 * HR], BF16, tag="w", name=f"vtq{q}")
                   for q in range(KT // 4)]
            zA = [zpool.tile([P, BC], BF16, tag="z", name=f"zA{j}")
                  for j in range(JT)]

            def wsl(tiles, kt, j):
                """[128,128] stationary slice for k-tile kt, out-feature j."""
                base = (kt % 2) * HID + j * P
                return tiles[kt // 2][:, base: base + P]

            # --- DMA issue plan (sync/gpsimd/scalar only, ~0.7us each) ------
            # The head is DMA-QUEUE-rate bound (~25 GB/s per queue, 16 queues
            # ~= 0.4 GB/us) and per-queue FIFO order is the cross-engine
            # ISSUE-TIME order of each transfer's descriptors.  W1 streams in
            # column-QUARTERS (jt blocks of 4) so arrival paces L1's jt-order
            # consumption: 1 MiB (x c0 + W1 q0) gates the first matmul at
            # ~11us, and each later 0.5 MiB piece lands ~2us before its
            # first reader.  Nothing bulky may enqueue before the waves.
            b1t = cpool.tile([P, JT], F32, tag="b1")
            nc.scalar.dma_start(b1t[:], B1)           # needed by first ACT

            def w1q_dma(k, qr, eng):
                o = (k % 2) * HID + qr * NB
                eng.dma_start(
                    w1p[k // 2][:, o: o + NB],
                    W1P[:, k * HID + qr * NB: k * HID + (qr + 1) * NB],
                )

            # wave 1a: x chunk c0 + W1 quarter q0 (enables L1 c0 jt0-3)
            w1q_dma(0, 0, nc.scalar)
            nc.sync.dma_start(xt[0][:, 0:NB], XP[:, 0 * BC: 0 * BC + NB])
            nc.gpsimd.dma_start(xt[1][:, 0:NB], XP[:, 1 * BC: 1 * BC + NB])
            nc.scalar.dma_start(xt[2][:, 0:NB], XP[:, 2 * BC: 2 * BC + NB])
            nc.sync.dma_start(xt[3][:, 0:NB], XP[:, 3 * BC: 3 * BC + NB])
            w1q_dma(1, 0, nc.sync)
            w1q_dma(2, 0, nc.gpsimd)
            w1q_dma(3, 0, nc.gpsimd)
            # W1 q1-q3 (jt4-15, needed ~17/20/24us): ALL W1 quarters stream
            # before x c1 -- x c1 isn't read until ~27us, and wedging its
            # 0.5 MiB mid-stream used to delay q2/q3 past their readers.
            w1q_dma(0, 1, nc.scalar)
            w1q_dma(1, 1, nc.sync)
            w1q_dma(2, 1, nc.gpsimd)
            w1q_dma(3, 1, nc.gpsimd)
            w1q_dma(0, 2, nc.scalar)
            w1q_dma(1, 2, nc.sync)
            w1q_dma(2, 2, nc.gpsimd)
            w1q_dma(3, 2, nc.gpsimd)
            w1q_dma(0, 3, nc.sync)
            w1q_dma(1, 3, nc.sync)
            w1q_dma(2, 3, nc.gpsimd)
            w1q_dma(3, 3, nc.gpsimd)
            # x c1 (needed ~27us, lands ~17us)
            nc.sync.dma_start(xt[0][:, NB:BC], XP[:, 0 * BC + NB: 1 * BC])
            nc.gpsimd.dma_start(xt[1][:, NB:BC], XP[:, 1 * BC + NB: 2 * BC])
            nc.scalar.dma_start(xt[2][:, NB:BC], XP[:, 2 * BC + NB: 3 * BC])
            nc.sync.dma_start(xt[3][:, NB:BC], XP[:, 3 * BC + NB: 4 * BC])

            # W2 entirely on sync, p0..p7 in order: its descriptors hit the
            # queues strictly first-half-first with nothing else wedged, so
            # p0-3 (gates L2-A at ~40us) complete ~27us.  gpsimd's bulk
            # (w3p4-7, vtq) all ring-block on pool slots that free at ~95us+
            # -- a blocked dma_start enqueues NO descriptors, keeping the
            # early queues clean.
            b2t = cpool.tile([P, JT], F32, tag="b2")
            nc.gpsimd.dma_start(b2t[:], B2)           # needed ~42us
            onest = cpool.tile([P, P], BF16, tag="ones")
            nc.gpsimd.memset(onest[:], 1.0)
            w0c = cpool.tile([P, HEADS], BF16, tag="w0c")
            nc.gpsimd.dma_start(w0c[:], W0C)
            for j in range(8):
                nc.sync.dma_start(w2p[j][:], W2P[:, j * 2 * HID: (j + 1) * 2 * HID])
            sqt = cpool.tile([P, KT * HEADS], BF16, tag="sq")
            nc.scalar.dma_start(sqt[:], SQ)
            b3t = cpool.tile([P, JT], F32, tag="b3")
            nc.sync.dma_start(b3t[:], B3)
            fwt = cpool.tile([P, KT * HEADS], BF16, tag="fw")
            nc.sync.dma_start(fwt[:], FW)
            # w3p0/p1 get fresh ring slots (descriptors follow W2's); p2/p3
            # block on sync until the W1 slots free (~44us); p4-7 block on
            # gpsimd until W2 pass-A slots free (~95us); vtq follows there.
            for j in range(4):
                nc.sync.dma_start(w3p[j][:], W3P[:, j * 2 * HID: (j + 1) * 2 * HID])
            for j in range(4, 8):
                nc.gpsimd.dma_start(w3p[j][:], W3P[:, j * 2 * HID: (j + 1) * 2 * HID])
            nc.gpsimd.dma_start(vtq[0][:], VTP[:, 0 * 4 * HR: 1 * 4 * HR])
            nc.gpsimd.dma_start(vtq[1][:], VTP[:, 1 * 4 * HR: 2 * 4 * HR])
            nc.gpsimd.dma_start(vtq[2][:], VTP[:, 2 * 4 * HR: 3 * 4 * HR])
            nc.gpsimd.dma_start(vtq[3][:], VTP[:, 3 * 4 * HR: 4 * 4 * HR])

            # --- embedder ----------------------------------------------------
            def layer1():
                """c-outer so chunk c0 only needs wave-1 data."""
                h_out = [hpool.tile([P, BC], BF16, tag="h", name=f"l1h{j}")
                         for j in range(JT)]
                for c in range(NBC):
                    for jt in range(JT):
                        nfill = 0
                        if c == 0 and nfill:
                            # dep-free fillers: execute only while the next
                            # group's trickling DMA pieces are late, keeping
                            # the PE busy so HAM doesn't re-throttle.
                            for _ in range(nfill):
                                nc.tensor.matmul(
                                    wu[:], warm[:, 0:P], warm[:],
                                    start=True, stop=True,
                                )
                        ps = pp.tile([P, NB], F32, tag="ps", name=f"l1ps{c}_{jt}")
                        kts = [(kt + jt) % KT1 for kt in range(KT1)]
                        for i, kt in enumerate(kts):
                            nc.tensor.matmul(
                                ps[:],
                                wsl(w1p, kt, jt),
                                xt[kt][:, c * NB: (c + 1) * NB],
                                start=(i == 0),
                                stop=(i == KT1 - 1),
                            )
                        nc.scalar.activation(
                            h_out[jt][:, c * NB: (c + 1) * NB],
                            ps[:],
                            AF.Tanh,
                            bias=b1t[:, jt: jt + 1],
                        )
                return h_out

            def layer2pass(h_prev, w_pairs, bias_t, name):
                """k-halved: pass A (kt 0-7) stages into bf16 zA, pass B
                (kt 8-15) accumulates, DVE folds zA in, ACT applies tanh."""
                # pass A
                for jt in range(JT):
                    ps = [pp.tile([P, NB], F32, tag="ps", name=f"{name}a{jt}_{c}")
                          for c in range(NBC)]
                    kts = [(kt + jt) % KH for kt in range(KH)]
                    for i, kt in enumerate(kts):
                        lhsT = wsl(w_pairs, kt, jt)
                        for c in range(NBC):
                            nc.tensor.matmul(
                                ps[c][:],
                                lhsT,
                                h_prev[kt][:, c * NB: (c + 1) * NB],
                                start=(i == 0),
                                stop=(i == KH - 1),
                            )
                    for c in range(NBC):
                        nc.vector.tensor_copy(
                            zA[jt][:, c * NB: (c + 1) * NB], ps[c][:]
                        )
                # pass B
                h_out = []
                for jt in range(JT):
                    ps = [pp.tile([P, NB], F32, tag="ps", name=f"{name}b{jt}_{c}")
                          for c in range(NBC)]
                    kts = [KH + (kt + jt) % KH for kt in range(KH)]
                    for i, kt in enumerate(kts):
                        lhsT = wsl(w_pairs, kt, jt)
                        for c in range(NBC):
                            nc.tensor.matmul(
                                ps[c][:],
                                lhsT,
                                h_prev[kt][:, c * NB: (c + 1) * NB],
                                start=(i == 0),
                                stop=(i == KH - 1),
                            )
                    ht = hpool.tile([P, BC], BF16, tag="h", name=f"{name}h{jt}")
                    for c in range(NBC):
                        nc.vector.tensor_add(
                            ps[c][:], ps[c][:], zA[jt][:, c * NB: (c + 1) * NB]
                        )
                        nc.scalar.activation(
                            ht[:, c * NB: (c + 1) * NB],
                            ps[c][:],
                            AF.Tanh,
                            bias=bias_t[:, jt: jt + 1],
                        )
                    h_out.append(ht)
                return h_out

            h1 = layer1()
            h2 = layer2pass(h1, w2p, b2t, "l2")
            h3 = layer2pass(h2, w3p, b3t, "l3")

            # --- h3 squared (stationary operand for the diag matmuls) -----
            h3sq = []
            for k in range(KT):
                sq_k = hpool.tile([P, BC], BF16, tag="h", name=f"h3sq{k}")
                nc.vector.tensor_mul(sq_k[:], h3[k][:], h3[k][:])
                h3sq.append(sq_k)

            # --- FM stage: per 128-row batch tile -------------------------
            def vsl(kt, half):
                """[128,512] moving slice of V^T for k-tile kt."""
                base = (kt % 4) * HR + half * NB
                return vtq[kt // 4][:, base: base + NB]

            def fm_phase_a(bt):
                """vx = h V^T (1024 cols) and lin = h fm_w^T (64 cols)."""
                vx0 = pp.tile([P, NB], F32, tag="ps", name=f"vx0_{bt}")
                vx1 = pp.tile([P, NB], F32, tag="ps", name=f"vx1_{bt}")
                lw = pp.tile([P, NB], F32, tag="ps", name=f"lw_{bt}")
                bsl = slice(bt * P, (bt + 1) * P)
                for kt in range(KT):
                    lhsT = h3[kt][:, bsl]
                    nc.tensor.matmul(
                        vx0[:], lhsT, vsl(kt, 0),
                        start=(kt == 0), stop=(kt == KT - 1),
                    )
                    nc.tensor.matmul(
                        vx1[:], lhsT, vsl(kt, 1),
                        start=(kt == 0), stop=(kt == KT - 1),
                    )
                    nc.tensor.matmul(
                        lw[:, 0:HEADS], lhsT,
                        fwt[:, kt * HEADS: (kt + 1) * HEADS],
                        start=(kt == 0), stop=(kt == KT - 1),
                    )
                return vx0, vx1, lw

            def fm_phase_b(bt):
                """diag = (h*h) . (0.5 * sum_r V^2), already scaled by 0.5."""
                dg = pp.tile([P, NB], F32, tag="ps", name=f"dg_{bt}")
                bsl = slice(bt * P, (bt + 1) * P)
                for kt in range(KT):
                    nc.tensor.matmul(
                        dg[:, 0:HEADS],
                        h3sq[kt][:, bsl],
                        sqt[:, kt * HEADS: (kt + 1) * HEADS],
                        start=(kt == 0), stop=False,
                    )
                nc.tensor.matmul(
                    dg[:, 0:HEADS], onest[:], w0c[:], start=False, stop=True,
                )
                return dg

            def fm_square_reduce(bt, vx0, vx1):
                """Emitted right after phase A: overlaps later bt's matmuls.
                Each 512-wide half squares then reduces independently so the
                two chains pipeline across ACT and DVE."""
                vx2 = epool.tile([P, HR], BF16, tag="e", name=f"vx2_{bt}")
                sumv = spool.tile([P, HEADS], F32, tag="s", name=f"sumv_{bt}")
                for c, vxh in ((0, vx0), (1, vx1)):
                    nc.scalar.activation(vx2[:, c * NB: (c + 1) * NB], vxh[:], AF.Square)
                    nc.vector.reduce_sum(
                        sumv[:, c * (HEADS // 2): (c + 1) * (HEADS // 2)],
                        vx2[:, c * NB: (c + 1) * NB].rearrange(
                            "p (h r) -> p h r", r=RANK
                        ),
                        axis=mybir.AxisListType.X,
                    )
                return sumv

            def fm_phase_a_chunked(bt):
                """Last batch tile only: vx in four 256-col chunks so each
                chunk's square+reduce drains DURING the next chunk's matmuls
                -- only ~0.6us of reduce work remains after the last matmul
                instead of the full 1.9us chain."""
                NCH = HR // 4
                lw = pp.tile([P, NB], F32, tag="ps", name=f"lw_{bt}")
                vx2 = epool.tile([P, HR], BF16, tag="e", name=f"vx2_{bt}")
                sumv = spool.tile([P, HEADS], F32, tag="s", name=f"sumv_{bt}")
                bsl = slice(bt * P, (bt + 1) * P)
                for ch in range(4):
                    vxc = pp.tile([P, NCH], F32, tag="ps", name=f"vxc{ch}_{bt}")
                    for kt in range(KT):
                        lhsT = h3[kt][:, bsl]
                        base = (kt % 4) * HR + ch * NCH
                        nc.tensor.matmul(
                            vxc[:], lhsT,
                            vtq[kt // 4][:, base: base + NCH],
                            start=(kt == 0), stop=(kt == KT - 1),
                        )
                        if ch == 0:
                            nc.tensor.matmul(
                                lw[:, 0:HEADS], lhsT,
                                fwt[:, kt * HEADS: (kt + 1) * HEADS],
                                start=(kt == 0), stop=(kt == KT - 1),
                            )
                    nc.scalar.activation(
                        vx2[:, ch * NCH: (ch + 1) * NCH], vxc[:], AF.Square
                    )
                    nc.vector.reduce_sum(
                        sumv[:, ch * (HEADS // 4): (ch + 1) * (HEADS // 4)],
                        vx2[:, ch * NCH: (ch + 1) * NCH].rearrange(
                            "p (h r) -> p h r", r=RANK
                        ),
                        axis=mybir.AxisListType.X,
                    )
                return sumv, lw

            ot = opool.tile([P, BT * HEADS], F32, tag="o")

            def fm_combine(bt, sumv, lw, dg):
                # q = 0.5*sumv - diag_half
                q = spool.tile([P, HEADS], F32, tag="s", name=f"q_{bt}")
                nc.vector.scalar_tensor_tensor(
                    q[:], sumv[:], 0.5, dg[:, 0:HEADS],
                    op0=ALU.mult, op1=ALU.subtract,
                )
                nc.vector.tensor_add(
                    ot[:, bt * HEADS: (bt + 1) * HEADS], q[:], lw[:, 0:HEADS]
                )
                # ship per-bt: gpsimd is idle through the FM stage, so only
                # the LAST tile's ~0.8us issue+transfer lands on the tail.
                nc.gpsimd.dma_start(
                    OUT[:, bt * HEADS: (bt + 1) * HEADS],
                    ot[:, bt * HEADS: (bt + 1) * HEADS],
                )

            # Stagger: A(0), A(1), B(0), C(0), A(2), B(1), C(1), ...
            pend = []  # (bt, sumv, lw)
            for bt in range(BT):
                if bt < BT - 1:
                    vx0, vx1, lw = fm_phase_a(bt)
                    sumv = fm_square_reduce(bt, vx0, vx1)
                else:
                    sumv, lw = fm_phase_a_chunked(bt)
                pend.append((bt, sumv, lw))
                if len(pend) == 2:
                    obt, osumv, olw = pend.pop(0)
                    dg = fm_phase_b(obt)
                    fm_combine(obt, osumv, olw, dg)
            while pend:
                obt, osumv, olw = pend.pop(0)
                dg = fm_phase_b(obt)
                fm_combine(obt, osumv, olw, dg)



    nc.compile()
    return nc


def _get_nc():
    if "nc" not in _CACHE:
        _CACHE["nc"] = _build_module()
    return _CACHE["nc"]


def _pack_rows(M, kt):
    """[kt*128, C] -> [128, kt*C] with [p, k*C+c] = M[k*128+p, c]."""
    C = M.shape[1]
    return np.ascontiguousarray(
        M.reshape(kt, P, C).transpose(1, 0, 2).reshape(P, kt * C)
    )


def _prep_host(x, W1, b1, W2, b2, W3, b3, fm_w0, fm_w, fm_V):
    """Host-side layout prep: bf16 casts, packing, per-head V reductions."""
    bf = ml_dtypes.bfloat16
    f32 = np.float32

    common = {
        "W1P": _pack_rows(W1.astype(bf), KT1),
        "W2P": _pack_rows(W2.astype(bf), KT),
        "W3P": _pack_rows(W3.astype(bf), KT),
        "B1": np.ascontiguousarray(b1.astype(f32).reshape(JT, P).T),
        "B2": np.ascontiguousarray(b2.astype(f32).reshape(JT, P).T),
        "B3": np.ascontiguousarray(b3.astype(f32).reshape(JT, P).T),
        # V^T: [2048, heads*rank] packed as [128, 16*1024]
        "VTP": _pack_rows(
            fm_V.reshape(HEADS * RANK, HID).T.astype(bf), KT
        ),
        # fm_w^T packed as [128, kt*64]: FW[p, kt*64+h] = fm_w[h, kt*128+p]
        "FW": np.ascontiguousarray(
            fm_w.T.reshape(KT, P, HEADS).transpose(1, 0, 2).reshape(P, KT * HEADS)
            .astype(bf)
        ),
        # 0.5 * sum_r V^2, same packing
        "SQ": np.ascontiguousarray(
            (0.5 * (fm_V.astype(np.float64) ** 2).sum(axis=1))
            .T.reshape(KT, P, HEADS).transpose(1, 0, 2).reshape(P, KT * HEADS)
            .astype(bf)
        ),
        "W0C": np.ascontiguousarray(
            np.tile((-fm_w0.astype(np.float64) / P)[None, :], (P, 1))
            .astype(ml_dtypes.bfloat16)
        ),
    }

    in_maps = []
    xb = x.astype(bf)
    for c in range(NCORES):
        m = dict(common)
        m["XP"] = _pack_rows(
            np.ascontiguousarray(xb[c * BC: (c + 1) * BC, :].T), KT1
        )
        in_maps.append(m)
    return in_maps


def kernel(x, W1, b1, W2, b2, W3, b3, fm_w0, fm_w, fm_V):
    # Host prep is plain numpy; coerce eagerly in case inputs are jax arrays.
    x, W1, b1, W2, b2, W3, b3, fm_w0, fm_w, fm_V = (
        np.asarray(a) for a in (x, W1, b1, W2, b2, W3, b3, fm_w0, fm_w, fm_V)
    )
    nc = _get_nc()
    in_maps = _prep_host(x, W1, b1, W2, b2, W3, b3, fm_w0, fm_w, fm_V)
    import os
    trace = bool(int(os.environ.get("KERNEL_TRACE", "0")))
    last_err = None
    for _attempt in range(3):
        try:
            res = bass_utils.run_bass_kernel_spmd(
                nc, in_maps, core_ids=list(range(NCORES)), trace=trace,
            )
            outs = [np.asarray(res.results[c]["out"]) for c in range(NCORES)]
            break
        except Exception as e:  # transient device faults (NRT unrecoverable)
            last_err = e
    else:
        raise last_err
    _CACHE["last_results"] = res
    # per-core out is [128, bt*64+c]; unpack to [BC, HEADS] then stack
    full = np.concatenate(
        [o.reshape(P, BT, HEADS).transpose(1, 0, 2).reshape(BC, HEADS)
         for o in outs],
        axis=0,
    )                                            # [B, HEADS]
    return np.ascontiguousarray(full.T).astype(np.float32)  # [HEADS, B]
